# revision 29
# baseline (speedup 1.0000x reference)
"""Trainium2 Bass kernel for a top-k BCE + soft-Dice loss.

Math
----
reference computes, over n = 9,437,184 elements:
  bce_map = softplus(x) - x*t          (elementwise, stable BCE-with-logits)
  bce     = mean(top_k(bce_map, k)),   k = int(0.2 * n)
  p       = sigmoid(x)
  dice    = (2*sum(p*t) + eps) / (sum(p) + sum(t) + eps)
  loss    = bce + 0.5*(1 - dice)

Key identity: for tau* = k-th largest of bce_map,
  sum_topk = k*tau* + sum(relu(bce_map - tau*))        (exact)
and the RHS is *second-order* insensitive to errors in tau, so a host-side
subsample estimate of tau lets the device compute the loss in one streaming
pass (no distributed top-k).

Device formulation (bf16 on device; sums accumulate in fp32).  The host
sends xn = -x so every device op needs only the negated logits:
  em   = sigmoid(xn)            ACT pass 1 (sigmoid table), accum -> sum(em)
  nspt = ln(em * e^tau)         ACT pass 2 (ln table) = -softplus(x) + tau
  xtn  = xn * t                 DVE tensor_tensor (2x bf16 mode)
  d    = xtn - nspt             DVE tensor_tensor (2x) = bce - tau
  r    = max(d, 0)              DVE tensor_scalar (4x)
  emt  = em * t                 DVE tensor_tensor (2x)
  PE   : ones^T @ {t, emt, r} -> column partial sums accumulated in PSUM
Host merges the tiny per-core partials in float64:
  bce  = tau + sum(r)/k;  sum(p) = n - sum(em);  sum(p*t) = sum(t) - sum(emt)

Schedule notes: the 8 cores share device HBM (~150-250 GB/s effective per
core), so the input stream paces the kernel start; ACT (two passes + two
table loads) and DVE (three products + relu) are the ~20us co-bottleneck
engines.  The first sigmoid tile is small so ACT starts early, the ln
phase runs in half-tile steps with the d/relu/reduce chain interleaved
right behind each step, and the ln order ends on the small tile so the
trailing chain is short.
"""

import os

import numpy as np

N_CORES = 8
P = 128
# sigma-phase tiles; tile 0 is small so the first sigmoid starts early
TILES = (512, 1792, 2304, 2304, 2304)
NT = len(TILES)
# ln-phase steps (suffixes of the same column space), processed in order;
# ends on the 512-column chunk for a short tail
LN_STEPS = (1152, 1152, 1152, 1152, 1152, 1152, 1280, 512, 512)
COLS = sum(TILES)                   # 9216 columns per core
SHARD = P * COLS                    # 1,179,648 elements per core
N_TOTAL = N_CORES * SHARD
TOPK_RATIO = 0.2
DICE_WEIGHT = 0.5
DICE_EPS = 1e-6

assert sum(LN_STEPS) == COLS

_BUILT = {}
LAST_RESULTS = None     # BassKernelResults of the most recent device run


def _build(ln_scale: float):
    """Trace the Bass/Tile program once; reuse across calls."""
    key = ("nc", round(float(ln_scale), 6))
    if key in _BUILT:
        return _BUILT[key]

    import concourse.tile as tile
    from concourse import bacc, mybir

    bf = mybir.dt.bfloat16
    f8 = mybir.dt.float8e4
    f32 = mybir.dt.float32
    Alu = mybir.AluOpType
    Act = mybir.ActivationFunctionType

    nc = bacc.Bacc("TRN2", target_bir_lowering=False, debug=False)
    xl = [nc.dram_tensor(f"xl{i}", [P, TILES[i]], bf, kind="ExternalInput")
          for i in range(NT)]
    tg = [nc.dram_tensor(f"tg{i}", [P, TILES[i]], bf, kind="ExternalInput")
          for i in range(NT)]
    sem = nc.dram_tensor("sem", [P, NT], f32, kind="ExternalOutput")   # sum(em)
    pes = nc.dram_tensor("pes", [1, 1536], f32, kind="ExternalOutput")  # t|emt|r

    with tile.TileContext(nc) as tc:
        with (
            tc.tile_pool(name="io", bufs=1) as io,
            tc.tile_pool(name="mid", bufs=1) as mid,
            tc.tile_pool(name="small", bufs=1) as small,
            tc.tile_pool(name="ppool", bufs=1, space="PSUM") as ppool,
        ):
            ones = small.tile([P, 1], bf)
            sem_sb = small.tile([P, NT], f32)
            pt_t = ppool.tile([1, 512], f32)
            pt_e = ppool.tile([1, 512], f32)
            pt_r = ppool.tile([1, 512], f32)

            # Whole-shard SBUF images of em and xtn, so the ln phase can
            # walk the column space independently of the sigma tiling.
            em_all = mid.tile([P, COLS], bf)
            xt_all = mid.tile([P, COLS], bf)

            offs = [sum(TILES[:i]) for i in range(NT)]
            xs, ts = [], []
            for i, fd in enumerate(TILES):
                xs.append(io.tile([P, fd], bf, tag=f"x{i}", name=f"x{i}"))
                ts.append(io.tile([P, fd], bf, tag=f"t{i}", name=f"t{i}"))

            # --- DMA: x/t interleaved per tile, issued from four different
            # engine queues so the ~0.65us-per-dma_start descriptor
            # generation does not serialize the transfers ---
            for i in range(NT):
                nc.sync.dma_start(out=xs[i][:], in_=xl[i].ap())
                nc.sync.dma_start(out=ts[i][:], in_=tg[i].ap())
                if i == 0:
                    nc.vector.memset(ones[:], 1.0)

            # --- ACT phase 1: sigmoid (first table load) ---
            for i in range(NT):
                nc.scalar.activation(
                    em_all[:, offs[i]:offs[i] + TILES[i]], xs[i][:],
                    Act.Sigmoid, accum_out=sem_sb[:, i:i + 1],
                )

            # --- DVE: products, in input-arrival order ---
            emts = []
            for i in range(NT):
                nc.vector.tensor_tensor(
                    xt_all[:, offs[i]:offs[i] + TILES[i]], xs[i][:], ts[i][:],
                    Alu.mult)
                emt = mid.tile([P, TILES[i]], bf, tag=f"emt{i}", name=f"emt{i}")
                nc.vector.tensor_tensor(
                    emt[:], em_all[:, offs[i]:offs[i] + TILES[i]], ts[i][:],
                    Alu.mult)
                emts.append(emt)

            # --- PE reduction helper: ones^T @ Y column sums into PSUM ---
            counters = {"t": 0, "e": 0, "r": 0}
            totals = {"t": COLS // 512, "e": COLS // 512,
                      "r": sum((w + 511) // 512 for w in LN_STEPS)}

            def reduce_cols(bank, key, src, width):
                for lo in range(0, width, 512):
                    hi = min(lo + 512, width)
                    nc.tensor.matmul(
                        bank[:, :hi - lo], ones[:], src[:, lo:hi],
                        start=(counters[key] == 0),
                        stop=(counters[key] == totals[key] - 1),
                    )
                    counters[key] += 1

            for i in range(NT):
                reduce_cols(pt_t, "t", ts[i][:], TILES[i])
                reduce_cols(pt_e, "e", emts[i][:], TILES[i])

            # --- ACT phase 2: ln (second table load) in steps over the
            # whole column space; d (DVE 2x), relu (DVE 4x) and the PE
            # reduce trail each step ---
            lo = 0
            for w in LN_STEPS:
                nsp = mid.tile([P, w], bf, tag="nsp", bufs=6, name="nsp")
                nc.scalar.activation(
                    nsp[:], em_all[:, lo:lo + w], Act.Ln, scale=ln_scale)
                d = mid.tile([P, w], bf, tag="d", bufs=3, name="d")
                nc.vector.tensor_tensor(
                    d[:], xt_all[:, lo:lo + w], nsp[:], Alu.subtract)
                r = mid.tile([P, w], bf, tag="r", bufs=3, name="r")
                nc.vector.tensor_scalar(r[:], d[:], 0.0, None, Alu.max)
                reduce_cols(pt_r, "r", r[:], w)
                lo += w

            # PSUM -> SBUF -> DRAM readout; t/emt copies overlap the
            # trailing d/relu work, the r copy is the true tail
            pes_sb = small.tile([1, 1536], f32)
            nc.scalar.copy(pes_sb[:, 0:512], pt_t[:, :])
            nc.scalar.copy(pes_sb[:, 512:1024], pt_e[:, :])
            nc.scalar.copy(pes_sb[:, 1024:1536], pt_r[:, :])
            nc.sync.dma_start(out=sem.ap(), in_=sem_sb[:])
            nc.sync.dma_start(out=pes.ap(), in_=pes_sb[:])

    nc.compile()
    _BUILT[key] = nc
    return nc


def _estimate_tau(xf, tf, k, n):
    """k-th largest of the BCE map, estimated from a strided subsample."""
    xs = xf[::7].astype(np.float64)
    ts = tf[::7].astype(np.float64)
    b = np.maximum(xs, 0.0) - xs * ts + np.log1p(np.exp(-np.abs(xs)))
    m = b.size
    kk = max(1, min(m, int(round(m * (k / n)))))
    return float(np.partition(b, m - kk)[m - kk])


def kernel(logits: np.ndarray, targets: np.ndarray) -> np.ndarray:
    global LAST_RESULTS
    import ml_dtypes
    from concourse import bass_utils

    xf = np.ascontiguousarray(logits, dtype=np.float32).reshape(-1)
    tf = np.ascontiguousarray(targets, dtype=np.float32).reshape(-1)
    n = xf.size
    assert n == N_TOTAL, f"kernel hardcoded for {N_TOTAL} elements, got {n}"
    k = max(1, int(n * TOPK_RATIO))

    tau = _estimate_tau(xf, tf, k, n)
    ln_scale = float(np.exp(tau))

    xsh = (-xf).astype(ml_dtypes.bfloat16).reshape(N_CORES, P, COLS)
    tsh = tf.astype(ml_dtypes.bfloat16).reshape(N_CORES, P, COLS)
    offs = [sum(TILES[:i]) for i in range(NT)]
    in_maps = []
    for c in range(N_CORES):
        m = {}
        for i, fd in enumerate(TILES):
            m[f"xl{i}"] = np.ascontiguousarray(xsh[c, :, offs[i]:offs[i] + fd])
            m[f"tg{i}"] = np.ascontiguousarray(tsh[c, :, offs[i]:offs[i] + fd])
        in_maps.append(m)

    nc = _build(ln_scale)
    trace = os.environ.get("KERNEL_TRACE", "0") == "1"
    res = bass_utils.run_bass_kernel_spmd(
        nc, in_maps, core_ids=list(range(N_CORES)), trace=trace,
    )
    LAST_RESULTS = res

    sum_em = 0.0
    sum_rl = 0.0
    sum_t = 0.0
    sum_emt = 0.0
    for r in res.results:
        sum_em += r["sem"].astype(np.float64).sum()
        pes = r["pes"].astype(np.float64)
        sum_t += pes[0, 0:512].sum()
        sum_emt += pes[0, 512:1024].sum()
        sum_rl += pes[0, 1024:1536].sum()

    bce_mean = tau + sum_rl / k
    sum_p = n - sum_em
    sum_pt = sum_t - sum_emt
    dice = (2.0 * sum_pt + DICE_EPS) / (sum_p + sum_t + DICE_EPS)
    loss = bce_mean + DICE_WEIGHT * (1.0 - dice)
    return np.array(loss, dtype=np.float32)


# revision 30
# speedup vs baseline: 1.0293x; 1.0293x over previous
"""Trainium2 Bass kernel for a top-k BCE + soft-Dice loss.

Math
----
reference computes, over n = 9,437,184 elements:
  bce_map = softplus(x) - x*t          (elementwise, stable BCE-with-logits)
  bce     = mean(top_k(bce_map, k)),   k = int(0.2 * n)
  p       = sigmoid(x)
  dice    = (2*sum(p*t) + eps) / (sum(p) + sum(t) + eps)
  loss    = bce + 0.5*(1 - dice)

Key identity: for tau* = k-th largest of bce_map,
  sum_topk = k*tau* + sum(relu(bce_map - tau*))        (exact)
and the RHS is *second-order* insensitive to errors in tau, so a host-side
subsample estimate of tau lets the device compute the loss in one streaming
pass (no distributed top-k).

Device formulation (bf16 on device; sums accumulate in fp32).  The host
sends xn = -x so every device op needs only the negated logits:
  em   = sigmoid(xn)            ACT pass 1 (sigmoid table), accum -> sum(em)
  nspt = ln(em * e^tau)         ACT pass 2 (ln table) = -softplus(x) + tau
  xtn  = xn * t                 DVE tensor_tensor (2x bf16 mode)
  d    = xtn - nspt             DVE tensor_tensor (2x) = bce - tau
  r    = max(d, 0)              DVE tensor_scalar (4x)
  emt  = em * t                 DVE tensor_tensor (2x)
  PE   : ones^T @ {t, emt, r} -> column partial sums accumulated in PSUM
Host merges the tiny per-core partials in float64:
  bce  = tau + sum(r)/k;  sum(p) = n - sum(em);  sum(p*t) = sum(t) - sum(emt)

Schedule notes: the 8 cores share device HBM (~150-250 GB/s effective per
core), so the input stream paces the kernel start; ACT (two passes + two
table loads) and DVE (three products + relu) are the ~20us co-bottleneck
engines.  The first sigmoid tile is small so ACT starts early, the ln
phase runs in half-tile steps with the d/relu/reduce chain interleaved
right behind each step, and the ln order ends on the small tile so the
trailing chain is short.
"""

import os

import numpy as np

N_CORES = 8
P = 128
# sigma-phase tiles; tile 0 is small so the first sigmoid starts early
TILES = (512, 1792, 2304, 2304, 2304)
NT = len(TILES)
# ln-phase steps (suffixes of the same column space), processed in order;
# ends on the 512-column chunk for a short tail
LN_STEPS = (1152, 1152, 1152, 1152, 1152, 1152, 1280, 512, 512)
COLS = sum(TILES)                   # 9216 columns per core
SHARD = P * COLS                    # 1,179,648 elements per core
N_TOTAL = N_CORES * SHARD
TOPK_RATIO = 0.2
DICE_WEIGHT = 0.5
DICE_EPS = 1e-6

assert sum(LN_STEPS) == COLS

_BUILT = {}
LAST_RESULTS = None     # BassKernelResults of the most recent device run


def _build(ln_scale: float):
    """Trace the Bass/Tile program once; reuse across calls."""
    key = ("nc", round(float(ln_scale), 6))
    if key in _BUILT:
        return _BUILT[key]

    import concourse.tile as tile
    from concourse import bacc, mybir

    bf = mybir.dt.bfloat16
    f32 = mybir.dt.float32
    Alu = mybir.AluOpType
    Act = mybir.ActivationFunctionType

    nc = bacc.Bacc("TRN2", target_bir_lowering=False, debug=False)
    xl = [nc.dram_tensor(f"xl{i}", [P, TILES[i]], bf, kind="ExternalInput")
          for i in range(NT)]
    tg = [nc.dram_tensor(f"tg{i}", [P, TILES[i]], bf, kind="ExternalInput")
          for i in range(NT)]
    sem = nc.dram_tensor("sem", [P, NT], f32, kind="ExternalOutput")   # sum(em)
    pes = nc.dram_tensor("pes", [1, 1536], f32, kind="ExternalOutput")  # t|emt|r

    with tile.TileContext(nc) as tc:
        with (
            tc.tile_pool(name="io", bufs=1) as io,
            tc.tile_pool(name="mid", bufs=1) as mid,
            tc.tile_pool(name="small", bufs=1) as small,
            tc.tile_pool(name="ppool", bufs=1, space="PSUM") as ppool,
        ):
            ones = small.tile([P, 1], bf)
            sem_sb = small.tile([P, NT], f32)
            pt_t = ppool.tile([1, 512], f32)
            pt_e = ppool.tile([1, 512], f32)
            pt_r = ppool.tile([1, 512], f32)

            # Whole-shard SBUF images of em and xtn, so the ln phase can
            # walk the column space independently of the sigma tiling.
            em_all = mid.tile([P, COLS], bf)
            xt_all = mid.tile([P, COLS], bf)

            offs = [sum(TILES[:i]) for i in range(NT)]
            xs, ts = [], []
            for i, fd in enumerate(TILES):
                xs.append(io.tile([P, fd], bf, tag=f"x{i}", name=f"x{i}"))
                ts.append(io.tile([P, fd], bf, tag=f"t{i}", name=f"t{i}"))

            # --- DMA: x/t interleaved per tile ---
            for i in range(NT):
                nc.sync.dma_start(out=xs[i][:], in_=xl[i].ap())
                nc.sync.dma_start(out=ts[i][:], in_=tg[i].ap())
                if i == 0:
                    nc.vector.memset(ones[:], 1.0)

            # --- ACT phase 1: sigmoid (first table load) ---
            for i in range(NT):
                nc.scalar.activation(
                    em_all[:, offs[i]:offs[i] + TILES[i]], xs[i][:],
                    Act.Sigmoid, accum_out=sem_sb[:, i:i + 1],
                )

            # --- DVE: products, in input-arrival order ---
            emts = []
            for i in range(NT):
                nc.vector.tensor_tensor(
                    xt_all[:, offs[i]:offs[i] + TILES[i]], xs[i][:], ts[i][:],
                    Alu.mult)
                emt = mid.tile([P, TILES[i]], bf, tag=f"emt{i}", name=f"emt{i}")
                nc.vector.tensor_tensor(
                    emt[:], em_all[:, offs[i]:offs[i] + TILES[i]], ts[i][:],
                    Alu.mult)
                emts.append(emt)

            # --- PE reduction helper: ones^T @ Y column sums into PSUM ---
            counters = {"t": 0, "e": 0, "r": 0}
            totals = {"t": COLS // 512, "e": COLS // 512,
                      "r": sum((w + 511) // 512 for w in LN_STEPS)}

            def reduce_cols(bank, key, src, width):
                for lo in range(0, width, 512):
                    hi = min(lo + 512, width)
                    nc.tensor.matmul(
                        bank[:, :hi - lo], ones[:], src[:, lo:hi],
                        start=(counters[key] == 0),
                        stop=(counters[key] == totals[key] - 1),
                    )
                    counters[key] += 1

            for i in range(NT):
                reduce_cols(pt_t, "t", ts[i][:], TILES[i])
                reduce_cols(pt_e, "e", emts[i][:], TILES[i])

            # --- ACT phase 2: ln (second table load) in steps over the
            # whole column space; d (DVE 2x), relu (DVE 4x) and the PE
            # reduce trail each step ---
            lo = 0
            for w in LN_STEPS:
                nsp = mid.tile([P, w], bf, tag="nsp", bufs=6, name="nsp")
                nc.scalar.activation(
                    nsp[:], em_all[:, lo:lo + w], Act.Ln, scale=ln_scale)
                d = mid.tile([P, w], bf, tag="d", bufs=3, name="d")
                nc.vector.tensor_tensor(
                    d[:], xt_all[:, lo:lo + w], nsp[:], Alu.subtract)
                r = mid.tile([P, w], bf, tag="r", bufs=3, name="r")
                nc.vector.tensor_scalar(r[:], d[:], 0.0, None, Alu.max)
                reduce_cols(pt_r, "r", r[:], w)
                lo += w

            # PSUM -> SBUF -> DRAM readout; t/emt copies overlap the
            # trailing d/relu work, the r copy is the true tail
            pes_sb = small.tile([1, 1536], f32)
            nc.scalar.copy(pes_sb[:, 0:512], pt_t[:, :])
            nc.scalar.copy(pes_sb[:, 512:1024], pt_e[:, :])
            nc.scalar.copy(pes_sb[:, 1024:1536], pt_r[:, :])
            nc.sync.dma_start(out=sem.ap(), in_=sem_sb[:])
            nc.sync.dma_start(out=pes.ap(), in_=pes_sb[:])

    nc.compile()
    _BUILT[key] = nc
    return nc


def _estimate_tau(xf, tf, k, n):
    """k-th largest of the BCE map, estimated from a strided subsample."""
    xs = xf[::7].astype(np.float64)
    ts = tf[::7].astype(np.float64)
    b = np.maximum(xs, 0.0) - xs * ts + np.log1p(np.exp(-np.abs(xs)))
    m = b.size
    kk = max(1, min(m, int(round(m * (k / n)))))
    return float(np.partition(b, m - kk)[m - kk])


def kernel(logits: np.ndarray, targets: np.ndarray) -> np.ndarray:
    global LAST_RESULTS
    import ml_dtypes
    from concourse import bass_utils

    xf = np.ascontiguousarray(logits, dtype=np.float32).reshape(-1)
    tf = np.ascontiguousarray(targets, dtype=np.float32).reshape(-1)
    n = xf.size
    assert n == N_TOTAL, f"kernel hardcoded for {N_TOTAL} elements, got {n}"
    k = max(1, int(n * TOPK_RATIO))

    tau = _estimate_tau(xf, tf, k, n)
    ln_scale = float(np.exp(tau))

    xsh = (-xf).astype(ml_dtypes.bfloat16).reshape(N_CORES, P, COLS)
    tsh = tf.astype(ml_dtypes.bfloat16).reshape(N_CORES, P, COLS)
    offs = [sum(TILES[:i]) for i in range(NT)]
    in_maps = []
    for c in range(N_CORES):
        m = {}
        for i, fd in enumerate(TILES):
            m[f"xl{i}"] = np.ascontiguousarray(xsh[c, :, offs[i]:offs[i] + fd])
            m[f"tg{i}"] = np.ascontiguousarray(tsh[c, :, offs[i]:offs[i] + fd])
        in_maps.append(m)

    nc = _build(ln_scale)
    trace = os.environ.get("KERNEL_TRACE", "0") == "1"
    res = bass_utils.run_bass_kernel_spmd(
        nc, in_maps, core_ids=list(range(N_CORES)), trace=trace,
    )
    LAST_RESULTS = res

    sum_em = 0.0
    sum_rl = 0.0
    sum_t = 0.0
    sum_emt = 0.0
    for r in res.results:
        sum_em += r["sem"].astype(np.float64).sum()
        pes = r["pes"].astype(np.float64)
        sum_t += pes[0, 0:512].sum()
        sum_emt += pes[0, 512:1024].sum()
        sum_rl += pes[0, 1024:1536].sum()

    bce_mean = tau + sum_rl / k
    sum_p = n - sum_em
    sum_pt = sum_t - sum_emt
    dice = (2.0 * sum_pt + DICE_EPS) / (sum_p + sum_t + DICE_EPS)
    loss = bce_mean + DICE_WEIGHT * (1.0 - dice)
    return np.array(loss, dtype=np.float32)
